# revision 26
# baseline (speedup 1.0000x reference)
"""Multi-head attention Bass kernel for Trainium2 (8 NeuronCores).

Problem: B=8, T=2048, C=256, H=8, D=32 MHA (dense, full softmax over T).
Sharding: data-parallel over batch -- core b computes batch b end-to-end,
no collectives.  Weights are replicated; per-core x slice is [T, C].

Per-core dataflow (v8):
  1. x arrives pre-transposed/pre-cast from the host as xT [C,T] bf16
     (plain DMAs, no on-device transpose).  wq pre-scaled by 1/sqrt(C)
     host-side; bias folded into wproj row 32 (ones-row trick).
  2. qT/kT [D,T] packed 4 heads per [128,T] tile via M=128 matmuls; v
     with an appended ones-column (v_ext [T,33], bf16) whose AV row
     yields the softmax denominators for free.
  3. Attention runs as 8 (pair, t-half) s-loops.  PSUM: av [P,1024] =
     2 banks + 3x sc [128,1024] = 6 banks.  The depth-3 scores pipeline
     feeds TWO exp engines concurrently -- ScalarE exact exp() for head A,
     DVE Schraudolph exp for head B (tensor_scalar int16(x*128/ln2 +
     16248) bit-cast to bf16; ~1.5% weight err, ~1e-2 output rel err
     total).  This dual drain is the key: a single-engine exp stream
     measures 2x slower end-to-end.  AV matmuls trail the scores by two
     s-iterations so the PE never stalls on the freshest exp chunk.
  4. Normalization: the [1,1024] denominator rows are reshaped to
     [128,16] via a DRAM hop and reciprocal'd WIDE on DVE (~0.2us; a
     single-lane [1,1024] DVE reciprocal costs ~20us on HW and wrecks
     the exp stream), shipped back, partition-broadcast by DMA, and
     multiplied into nout on the otherwise-idle GpSimd.  All of it is
     deferred into the NEXT half's s-loop (popped at s==3), so only the
     two PSUM-freeing copies sit at the boundary.
  5. Projection: res = sum_pairs noutT_p.T @ wproj_p (bias rides the
     ones-row in nout[0]); output DMAs batched on the ScalarE HWDGE
     queue.
"""

import numpy as np
import ml_dtypes
from contextlib import ExitStack

import concourse.bass as bass
import concourse.bacc as bacc
import concourse.mybir as mybir
import concourse.tile as tile
from concourse.bass_utils import run_bass_kernel_spmd

B, T, C, H, D = 8, 2048, 256, 8, 32
P = 128
NT = T // P  # 16 chunks of 128 along t / s
F32 = mybir.dt.float32
BF16 = mybir.dt.bfloat16
I16 = mybir.dt.int16
EXP = mybir.ActivationFunctionType.Exp
AL = mybir.AluOpType
N_CORES = 8
E = D + 1  # 33: v columns + ones column

# Schraudolph exp on DVE: bf16 bits of exp(x) ~= int16(x*128/ln2 + 127*128-c).
# HW convert rounds to nearest (CoreSim truncates; both ~1.5% rel err on the
# weights, ~9.6e-3 on the output at a 50% chunk share).
_LN2 = float(np.log(2.0))
SCH_A = 128.0 / _LN2
SCH_B = 127.0 * 128.0 - 8.0


def _body(nc, tc, ctx, x_d, wq_d, wk_d, wv_d, wp_d, bias_d, out_d):
    const = ctx.enter_context(tc.tile_pool(name="const", bufs=1))
    big = ctx.enter_context(tc.tile_pool(name="big", bufs=1))

    wq_sb = const.tile([P, 2, C], BF16)
    wk_sb = const.tile([P, 2, C], BF16)
    wv_sb = const.tile([P, 2, C], BF16)
    wp_sb = const.tile([P, 4, C], BF16)
    ones_sb = const.tile([P, D], BF16)
    warm = const.tile([P, 1], F32)

    xT = [big.tile([P, T], BF16, name=f"xT{i}") for i in range(2)]
    qT = [big.tile([P, T], BF16, name=f"qT{i}") for i in range(2)]
    kT = [big.tile([P, T], BF16, name=f"kT{i}") for i in range(2)]
    v_sb = big.tile([P, NT, E * H], BF16)
    nout = [big.tile([P, T], BF16, name=f"nout{i}") for i in range(4)]

    # ---- Phase 1: x arrives pre-transposed from the host as xT [C,T];
    # plain DMAs only.  bf16 weights DMA'd straight into SBUF -------------
    for cc, eng in ((0, nc.scalar), (1, nc.sync)):
        eng.dma_start(out=xT[cc], in_=x_d[cc * P:(cc + 1) * P, :])
    for w_sb, w_d, nk in ((wq_sb, wq_d, 2), (wk_sb, wk_d, 2),
                          (wv_sb, wv_d, 2), (wp_sb, wp_d, 4)):
        if nk == 2:
            nc.scalar.dma_start(
                out=w_sb, in_=w_d.rearrange("(k p) c -> p k c", p=P))
        else:
            nc.scalar.dma_start(out=w_sb, in_=w_d.rearrange("q p c -> p q c"))
    nc.gpsimd.memset(v_sb, 1.0)  # ones cols survive; v overwrites the rest
    for t_ in nout:  # rows 32-63 / 96-127 must be 0.0 for the projection
        nc.gpsimd.memset(t_, 0.0)
    # ones-row at nout[0] row 32 picks up the bias row folded into wp[0]
    nc.vector.memset(nout[0][D:D + 1, :], 1.0)
    nc.vector.memset(ones_sb, 1.0)
    nc.scalar.activation(out=warm, in_=ones_sb[:, 0:1], func=EXP)

    # ---- Phase 2: v (warms the PE), then qT / kT (M=128 matmuls) --------
    with tc.tile_pool(name="pv", bufs=2, space="PSUM") as pv:
        for n in range(NT):
            vp = pv.tile([P, C], F32, tag="vp", name="vp")
            for cc in range(2):
                nc.tensor.matmul(
                    vp,
                    lhsT=xT[cc][:, n * P:(n + 1) * P],
                    rhs=wv_sb[:, cc, :],
                    start=(cc == 0), stop=(cc == 1))
            nc.vector.tensor_copy(
                v_sb[:, n].rearrange("p (h e) -> p h e", e=E)[:, :, 0:D],
                vp.rearrange("p (h d) -> p h d", d=D))
    with tc.tile_pool(name="pq", bufs=2, space="PSUM") as pq:
        for g in range(2):
            for w_sb, dest in ((wq_sb, qT), (wk_sb, kT)):
                qp = pq.tile([P, T], F32, tag="qp", name="qp")
                for ts in range(4):
                    for cc in range(2):
                        nc.tensor.matmul(
                            qp[:, 512 * ts:512 * (ts + 1)],
                            lhsT=w_sb[:, cc, P * g:P * (g + 1)],
                            rhs=xT[cc][:, 512 * ts:512 * (ts + 1)],
                            start=(cc == 0), stop=(cc == 1))
                if g == 0:  # ScalarE is idle pre-attention: fastest start
                    nc.scalar.copy(dest[g], qp)
                else:
                    nc.vector.tensor_copy(dest[g], qp)

    # ---- Phase 3: attention; (pair, t-half) loops ------------------------
    # PSUM: av [P,1024] = 2 banks (one t-half) + 3x sc [P,1024] = 6 banks.
    # The depth-3 scores pipeline lets ScalarE (exact exp, head A) and DVE
    # (Schraudolph int16 exp, head B) drain chunks concurrently; AV matmuls
    # trail the scores by a full s-iteration so the PE never stalls on the
    # freshest exp.  Normalization of each half is deferred into the next
    # half's s-loop (only the PSUM-freeing copies happen at the boundary).
    with tc.tile_pool(name="scp", bufs=3, space="PSUM") as scp, \
         tc.tile_pool(name="avp", bufs=1, space="PSUM") as avp, \
         tc.tile_pool(name="expp", bufs=10) as expp, \
         tc.tile_pool(name="nrm", bufs=2) as nrm, \
         tc.tile_pool(name="dsc", bufs=4, space="DRAM") as dscp:
        deferred = []

        def sc_chunk(g, h, half, s, use_dve):
            j = h % 4
            sc = scp.tile([P, 1024], F32, tag="sc", name="sc")
            for ts in range(2):
                tofs = 1024 * half + 512 * ts
                nc.tensor.matmul(
                    sc[:, 512 * ts:512 * (ts + 1)],
                    lhsT=kT[g][D * j:D * (j + 1), P * s:P * (s + 1)],
                    rhs=qT[g][D * j:D * (j + 1), tofs:tofs + 512],
                    start=True, stop=True,
                    tile_position=(D * j, 0))
            ex = expp.tile([P, 1024], BF16, tag="ex", name="ex")
            if use_dve:
                nc.vector.tensor_scalar(
                    ex[:, :].bitcast(I16), sc, SCH_A, SCH_B, AL.mult, AL.add)
            else:
                nc.scalar.activation(out=ex, in_=sc, func=EXP)
            return ex

        for pair in range(4):
            g = pair // 2
            hA, hB = 2 * pair, 2 * pair + 1
            avf = nrm.tile([P, T], F32, tag="avf", name="avf")
            bc = nrm.tile([P, T], F32, tag="bc", name="bc")
            for half in range(2):
                last = (pair == 3 and half == 1)
                cl, ch = 1024 * half, 1024 * (half + 1)
                av = avp.tile([P, 1024], F32, tag="av", name="av")

                def av_mm(s, exs):
                    for h in (hA, hB):
                        col = 0 if h == hA else 64
                        ex = exs.pop((s, h))
                        for ts in range(2):
                            # A and B share banks at different partition
                            # ranges; per-element has_written makes that safe
                            # on HW, but the sim's bank-granular group
                            # tracker would flag it.
                            nc.tensor.matmul(
                                av[col:col + E, 512 * ts:512 * (ts + 1)],
                                lhsT=v_sb[:, s, E * h:E * h + E],
                                rhs=ex[:, 512 * ts:512 * (ts + 1)],
                                start=(s == 0), stop=(s == NT - 1),
                                tile_position=(0, col),
                                skip_group_check=True)

                exs = {}
                for h, dve in ((hA, False), (hB, True)):
                    exs[(0, h)] = sc_chunk(g, h, half, 0, dve)
                for s in range(NT):
                    if s + 1 < NT:
                        for h, dve in ((hA, False), (hB, True)):
                            exs[(s + 1, h)] = sc_chunk(g, h, half, s + 1, dve)
                    if s >= 2:
                        av_mm(s - 2, exs)
                    if s == 5 and deferred:
                        deferred.pop(0)()
                av_mm(NT - 2, exs)
                av_mm(NT - 1, exs)

                # Evacuate the AV banks promptly: low rows on ScalarE,
                # high rows on DVE, in parallel.
                nc.scalar.copy(avf[0:E, cl:ch], av[0:E, :])
                nc.vector.tensor_copy(avf[64:64 + E, cl:ch],
                                      av[64:64 + E, :])

                # A [1,1024] DVE reciprocal is a single-lane iterative divide
                # (~20us on HW!), so reshape the two denominator rows to
                # [128,16] via a DRAM hop, take the reciprocal wide (~0.2us),
                # and ship back.  The inbound DMAs are issued here at the
                # boundary (sync queue, block nothing); only the reciprocal
                # and the outbound hops are deferred into the next half's
                # s-loop so the DVE never waits on the DMA chain.
                dsc = dscp.tile([2, 1024], F32, tag="dsc", name="dsc")
                drec = nrm.tile([P, 16], F32, tag="drec", name="drec")
                for i, row in enumerate((D, 64 + D)):
                    nc.sync.dma_start(out=dsc[i:i + 1, :],
                                      in_=avf[row:row + 1, cl:ch])
                    nc.sync.dma_start(
                        out=drec[:, 8 * i:8 * (i + 1)],
                        in_=dsc[i, :].rearrange("(p f) -> p f", p=P))

                def mk(cl=cl, ch=ch, avf=avf, bc=bc, pair=pair, drec=drec):
                    def go():
                        dsc2 = dscp.tile([2, 1024], F32, tag="ds2",
                                         name="dsc2")
                        nc.vector.reciprocal(out=drec[:, :], in_=drec[:, :])
                        for i in range(2):
                            nc.sync.dma_start(
                                out=dsc2[i, :].rearrange("(p f) -> p f", p=P),
                                in_=drec[:, 8 * i:8 * (i + 1)])
                        for i, base in enumerate((0, 64)):
                            nc.sync.dma_start(
                                out=bc[base:base + D, cl:ch],
                                in_=dsc2[i, :].partition_broadcast(D))
                            # SBUF-only multiply -> GpSimd keeps DVE free
                            nc.gpsimd.tensor_mul(
                                nout[pair][base:base + D, cl:ch],
                                avf[base:base + D, cl:ch],
                                bc[base:base + D, cl:ch])
                    return go
                if not last:
                    deferred.append(mk())
                else:
                    mk()()
        # run the remaining deferred normalization (pair 3 half 0)
        for go in deferred:
            go()

    # ---- Phase 4: output projection (bias pre-folded into wp row 32 via
    # the ones-row in nout[0]).  n-blocks 0-7 need only the early nout
    # columns; for n-blocks 8-15 the q=0..2 partials are accumulated into
    # open PSUM groups while the final denominator chain (which gates
    # nout[3] cols 1024+) is still in flight, keeping the PE busy and its
    # p-state up; only the q=3 matmuls run after it lands. ----------------
    with tc.tile_pool(name="prp", bufs=8, space="PSUM") as prp, \
         tc.tile_pool(name="resp", bufs=1) as resp:
        resbig = resp.tile([P, NT, C], F32, name="resbig")
        out_r = out_d.rearrange("(n p) c -> n p c", p=P)
        rps = {}
        for n in range(NT):
            rp = prp.tile([P, C], F32, tag="rp", name="rp")
            rps[n] = rp
            if n < 8:
                for q in range(4):
                    nc.tensor.matmul(
                        rp,
                        lhsT=nout[q][:, P * n:P * (n + 1)],
                        rhs=wp_sb[:, q, :],
                        start=(q == 0), stop=(q == 3))
                nc.vector.tensor_copy(resbig[:, n, :], rp)
            else:
                for q in range(3):
                    nc.tensor.matmul(
                        rp,
                        lhsT=nout[q][:, P * n:P * (n + 1)],
                        rhs=wp_sb[:, q, :],
                        start=(q == 0), stop=False)
        nc.scalar.dma_start(
            out=out_r[0:8].rearrange("n p c -> p n c"),
            in_=resbig[:, 0:8, :])
        for n in range(8, NT):
            nc.tensor.matmul(
                rps[n],
                lhsT=nout[3][:, P * n:P * (n + 1)],
                rhs=wp_sb[:, 3, :],
                start=False, stop=True)
            nc.vector.tensor_copy(resbig[:, n, :], rps[n])
            if n == 11:
                nc.sync.dma_start(
                    out=out_r[8:12].rearrange("n p c -> p n c"),
                    in_=resbig[:, 8:12, :])
            elif n == 14:
                nc.scalar.dma_start(
                    out=out_r[12:15].rearrange("n p c -> p n c"),
                    in_=resbig[:, 12:15, :])
            elif n == 15:
                nc.sync.dma_start(out=out_r[15], in_=resbig[:, 15, :])


def declare_io(nc):
    """Declare the kernel's DRAM IO tensors (shared with test harnesses)."""
    x_d = nc.dram_tensor("x", [C, T], BF16, kind="ExternalInput")
    wq_d = nc.dram_tensor("wq", [C, C], BF16, kind="ExternalInput")
    wk_d = nc.dram_tensor("wk", [C, C], BF16, kind="ExternalInput")
    wv_d = nc.dram_tensor("wv", [C, C], BF16, kind="ExternalInput")
    wp_d = nc.dram_tensor("wp", [4, P, C], BF16, kind="ExternalInput")
    bias_d = nc.dram_tensor("bias", [P, C], F32, kind="ExternalInput")
    out_d = nc.dram_tensor("out", [T, C], F32, kind="ExternalOutput")
    return x_d, wq_d, wk_d, wv_d, wp_d, bias_d, out_d


def build_nc():
    nc = bacc.Bacc("TRN2", debug=False, num_devices=N_CORES)
    x_d, wq_d, wk_d, wv_d, wp_d, bias_d, out_d = declare_io(nc)
    with tile.TileContext(nc) as tc:
        with ExitStack() as ctx:
            _body(nc, tc, ctx, x_d.ap(), wq_d.ap(), wk_d.ap(), wv_d.ap(),
                  wp_d.ap(), bias_d.ap(), out_d.ap())
    nc.compile()
    return nc


def prep_inputs(x, wq, wk, wv, wproj, bproj):
    """Host-side reformatting of the full inputs into per-core input maps."""
    f = np.float32
    bf = ml_dtypes.bfloat16
    # [H,C,D] -> [C, H*D]; wq additionally pre-scaled by 1/sqrt(C) (exact).
    wq2 = np.ascontiguousarray(
        np.transpose(np.asarray(wq, f), (1, 0, 2)).reshape(C, H * D)
        * f(1.0 / 16.0)).astype(bf)
    wk2 = np.ascontiguousarray(
        np.transpose(np.asarray(wk, f), (1, 0, 2)).reshape(C, H * D)).astype(bf)
    wv2 = np.ascontiguousarray(
        np.transpose(np.asarray(wv, f), (1, 0, 2)).reshape(C, H * D)).astype(bf)
    # wproj [H*D, C] -> 4 pair-chunks padded to 128 rows:
    # rows 0-31 <- head 2p, rows 64-95 <- head 2p+1, rest zero.
    wp4 = np.zeros((4, P, C), f)
    wproj = np.asarray(wproj, f)
    for p in range(4):
        wp4[p, 0:D] = wproj[64 * p: 64 * p + D]
        wp4[p, 64:64 + D] = wproj[64 * p + D: 64 * p + 2 * D]
    # bias rides row 32 of chunk 0 (ones-row in nout[0] row 32 on-device)
    wp4[0, D] = np.asarray(bproj, f)
    wp4 = wp4.astype(bf)
    bias128 = np.ascontiguousarray(
        np.broadcast_to(np.asarray(bproj, f), (P, C)))
    x_bf = np.asarray(x, f).astype(ml_dtypes.bfloat16)
    in_maps = []
    for b in range(N_CORES):
        in_maps.append({
            "x": np.ascontiguousarray(x_bf[b].T),  # host-side transpose
            "wq": wq2, "wk": wk2, "wv": wv2,
            "wp": wp4, "bias": bias128,
        })
    return in_maps


_NC_CACHE = []


def kernel(x, wq, wk, wv, wproj, bproj, _nc=None):
    in_maps = prep_inputs(x, wq, wk, wv, wproj, bproj)
    if _nc is None:
        if not _NC_CACHE:
            _NC_CACHE.append(build_nc())
        _nc = _NC_CACHE[0]
    res = run_bass_kernel_spmd(_nc, in_maps, list(range(N_CORES)))
    return np.stack([r["out"] for r in res.results], axis=0)



# revision 28
# speedup vs baseline: 1.0962x; 1.0962x over previous
"""Multi-head attention Bass kernel for Trainium2 (8 NeuronCores).

Problem: B=8, T=2048, C=256, H=8, D=32 MHA (dense, full softmax over T).
Sharding: data-parallel over batch -- core b computes batch b end-to-end,
no collectives.  Weights are replicated; per-core x slice is [T, C].

Per-core dataflow (v8):
  1. x arrives pre-transposed/pre-cast from the host as xT [C,T] bf16
     (plain DMAs, no on-device transpose).  wq pre-scaled by 1/sqrt(C)
     host-side; bias folded into wproj row 32 (ones-row trick).
  2. qT/kT [D,T] packed 4 heads per [128,T] tile via M=128 matmuls; v
     with an appended ones-column (v_ext [T,33], bf16) whose AV row
     yields the softmax denominators for free.
  3. Attention runs as 8 (pair, t-half) s-loops.  PSUM: av [P,1024] =
     2 banks + 3x sc [128,1024] = 6 banks.  The depth-3 scores pipeline
     feeds TWO exp engines concurrently -- ScalarE exact exp() for head A,
     DVE Schraudolph exp for head B (tensor_scalar int16(x*128/ln2 +
     16248) bit-cast to bf16; ~1.5% weight err, ~1e-2 output rel err
     total).  This dual drain is the key: a single-engine exp stream
     measures 2x slower end-to-end.  AV matmuls trail the scores by two
     s-iterations so the PE never stalls on the freshest exp chunk.
  4. Normalization: the [1,1024] denominator rows are reshaped to
     [128,16] via a DRAM hop and reciprocal'd WIDE on DVE (~0.2us; a
     single-lane [1,1024] DVE reciprocal costs ~20us on HW and wrecks
     the exp stream), shipped back, partition-broadcast by DMA, and
     multiplied into nout on the otherwise-idle GpSimd.  All of it is
     deferred into the NEXT half's s-loop (popped at s==3), so only the
     two PSUM-freeing copies sit at the boundary.
  5. Projection: res = sum_pairs noutT_p.T @ wproj_p (bias rides the
     ones-row in nout[0]); output DMAs batched on the ScalarE HWDGE
     queue.
"""

import numpy as np
import ml_dtypes
from contextlib import ExitStack

import concourse.bass as bass
import concourse.bacc as bacc
import concourse.mybir as mybir
import concourse.tile as tile
from concourse.bass_utils import run_bass_kernel_spmd

B, T, C, H, D = 8, 2048, 256, 8, 32
P = 128
NT = T // P  # 16 chunks of 128 along t / s
F32 = mybir.dt.float32
BF16 = mybir.dt.bfloat16
I16 = mybir.dt.int16
EXP = mybir.ActivationFunctionType.Exp
AL = mybir.AluOpType
N_CORES = 8
E = D + 1  # 33: v columns + ones column

# Schraudolph exp on DVE: bf16 bits of exp(x) ~= int16(x*128/ln2 + 127*128-c).
# HW convert rounds to nearest (CoreSim truncates; both ~1.5% rel err on the
# weights, ~9.6e-3 on the output at a 50% chunk share).
_LN2 = float(np.log(2.0))
SCH_A = 128.0 / _LN2
SCH_B = 127.0 * 128.0 - 8.0


def _body(nc, tc, ctx, x_d, wq_d, wk_d, wv_d, wp_d, bias_d, out_d):
    const = ctx.enter_context(tc.tile_pool(name="const", bufs=1))
    big = ctx.enter_context(tc.tile_pool(name="big", bufs=1))

    wq_sb = const.tile([P, 2, C], BF16)
    wk_sb = const.tile([P, 2, C], BF16)
    wv_sb = const.tile([P, 2, C], BF16)
    wp_sb = const.tile([P, 4, C], BF16)
    ones_sb = const.tile([P, D], BF16)
    warm = const.tile([P, 1], F32)

    xT = [big.tile([P, T], BF16, name=f"xT{i}") for i in range(2)]
    qT = [big.tile([P, T], BF16, name=f"qT{i}") for i in range(2)]
    kT = [big.tile([P, T], BF16, name=f"kT{i}") for i in range(2)]
    v_sb = big.tile([P, NT, E * H], BF16)
    nout = [big.tile([P, T], BF16, name=f"nout{i}") for i in range(4)]

    # ---- Phase 1: x arrives pre-transposed from the host as xT [C,T];
    # plain DMAs only.  bf16 weights DMA'd straight into SBUF -------------
    for cc, eng in ((0, nc.scalar), (1, nc.sync)):
        eng.dma_start(out=xT[cc], in_=x_d[cc * P:(cc + 1) * P, :])
    for w_sb, w_d, nk in ((wq_sb, wq_d, 2), (wk_sb, wk_d, 2),
                          (wv_sb, wv_d, 2), (wp_sb, wp_d, 4)):
        if nk == 2:
            nc.scalar.dma_start(
                out=w_sb, in_=w_d.rearrange("(k p) c -> p k c", p=P))
        else:
            nc.scalar.dma_start(out=w_sb, in_=w_d.rearrange("q p c -> p q c"))
    nc.gpsimd.memset(v_sb, 1.0)  # ones cols survive; v overwrites the rest
    for t_ in nout:  # rows 32-63 / 96-127 must be 0.0 for the projection
        nc.gpsimd.memset(t_, 0.0)
    # ones-row at nout[0] row 32 picks up the bias row folded into wp[0]
    nc.vector.memset(nout[0][D:D + 1, :], 1.0)
    nc.vector.memset(ones_sb, 1.0)
    nc.scalar.activation(out=warm, in_=ones_sb[:, 0:1], func=EXP)

    # ---- Phase 2: v (warms the PE), then qT / kT (M=128 matmuls) --------
    with tc.tile_pool(name="pv", bufs=2, space="PSUM") as pv:
        for n in range(NT):
            vp = pv.tile([P, C], F32, tag="vp", name="vp")
            for cc in range(2):
                nc.tensor.matmul(
                    vp,
                    lhsT=xT[cc][:, n * P:(n + 1) * P],
                    rhs=wv_sb[:, cc, :],
                    start=(cc == 0), stop=(cc == 1))
            nc.vector.tensor_copy(
                v_sb[:, n].rearrange("p (h e) -> p h e", e=E)[:, :, 0:D],
                vp.rearrange("p (h d) -> p h d", d=D))
    with tc.tile_pool(name="pq", bufs=2, space="PSUM") as pq:
        for g in range(2):
            for w_sb, dest in ((wq_sb, qT), (wk_sb, kT)):
                qp = pq.tile([P, T], F32, tag="qp", name="qp")
                for ts in range(4):
                    for cc in range(2):
                        nc.tensor.matmul(
                            qp[:, 512 * ts:512 * (ts + 1)],
                            lhsT=w_sb[:, cc, P * g:P * (g + 1)],
                            rhs=xT[cc][:, 512 * ts:512 * (ts + 1)],
                            start=(cc == 0), stop=(cc == 1))
                if g == 0:  # ScalarE is idle pre-attention: fastest start
                    nc.scalar.copy(dest[g], qp)
                else:
                    nc.vector.tensor_copy(dest[g], qp)

    # ---- Phase 3: attention; (pair, t-half) loops ------------------------
    # PSUM: av [P,1024] = 2 banks (one t-half) + 3x sc [P,1024] = 6 banks.
    # The depth-3 scores pipeline lets ScalarE (exact exp, head A) and DVE
    # (Schraudolph int16 exp, head B) drain chunks concurrently; AV matmuls
    # trail the scores by a full s-iteration so the PE never stalls on the
    # freshest exp.  Normalization of each half is deferred into the next
    # half's s-loop (only the PSUM-freeing copies happen at the boundary).
    with tc.tile_pool(name="scp", bufs=3, space="PSUM") as scp, \
         tc.tile_pool(name="avp", bufs=1, space="PSUM") as avp, \
         tc.tile_pool(name="expp", bufs=10) as expp, \
         tc.tile_pool(name="nrm", bufs=2) as nrm, \
         tc.tile_pool(name="dsc", bufs=4, space="DRAM") as dscp:
        deferred = []

        def sc_chunk(g, h, half, s, use_dve):
            j = h % 4
            sc = scp.tile([P, 1024], F32, tag="sc", name="sc")
            for ts in range(2):
                tofs = 1024 * half + 512 * ts
                nc.tensor.matmul(
                    sc[:, 512 * ts:512 * (ts + 1)],
                    lhsT=kT[g][D * j:D * (j + 1), P * s:P * (s + 1)],
                    rhs=qT[g][D * j:D * (j + 1), tofs:tofs + 512],
                    start=True, stop=True,
                    tile_position=(D * j, 0))
            ex = expp.tile([P, 1024], BF16, tag="ex", name="ex")
            if use_dve:
                nc.vector.tensor_scalar(
                    ex[:, :].bitcast(I16), sc, SCH_A, SCH_B, AL.mult, AL.add)
            else:
                nc.scalar.activation(out=ex, in_=sc, func=EXP)
            return ex

        for pair in range(4):
            g = pair // 2
            hA, hB = 2 * pair, 2 * pair + 1
            avf = nrm.tile([P, T], F32, tag="avf", name="avf")
            bc = nrm.tile([P, T], F32, tag="bc", name="bc")
            for half in range(2):
                last = (pair == 3 and half == 1)
                cl, ch = 1024 * half, 1024 * (half + 1)
                av = avp.tile([P, 1024], F32, tag="av", name="av")

                def av_mm(s, exs):
                    for h in (hA, hB):
                        col = 0 if h == hA else 64
                        ex = exs.pop((s, h))
                        for ts in range(2):
                            # A and B share banks at different partition
                            # ranges; per-element has_written makes that safe
                            # on HW, but the sim's bank-granular group
                            # tracker would flag it.
                            nc.tensor.matmul(
                                av[col:col + E, 512 * ts:512 * (ts + 1)],
                                lhsT=v_sb[:, s, E * h:E * h + E],
                                rhs=ex[:, 512 * ts:512 * (ts + 1)],
                                start=(s == 0), stop=(s == NT - 1),
                                tile_position=(0, col),
                                skip_group_check=True)

                exs = {}
                for h, dve in ((hA, False), (hB, True)):
                    exs[(0, h)] = sc_chunk(g, h, half, 0, dve)
                for s in range(NT):
                    if s + 1 < NT:
                        for h, dve in ((hA, False), (hB, True)):
                            exs[(s + 1, h)] = sc_chunk(g, h, half, s + 1, dve)
                    if s >= 3:
                        av_mm(s - 3, exs)
                    if s == 5 and deferred:
                        deferred.pop(0)()
                av_mm(NT - 3, exs)
                av_mm(NT - 2, exs)
                av_mm(NT - 1, exs)

                # Evacuate the AV banks promptly: low rows on ScalarE,
                # high rows on DVE, in parallel.
                nc.scalar.copy(avf[0:E, cl:ch], av[0:E, :])
                nc.vector.tensor_copy(avf[64:64 + E, cl:ch],
                                      av[64:64 + E, :])

                # A [1,1024] DVE reciprocal is a single-lane iterative divide
                # (~20us on HW!), so reshape the two denominator rows to
                # [128,16] via a DRAM hop, take the reciprocal wide (~0.2us),
                # and ship back.  The inbound DMAs are issued here at the
                # boundary (sync queue, block nothing); only the reciprocal
                # and the outbound hops are deferred into the next half's
                # s-loop so the DVE never waits on the DMA chain.
                dsc = dscp.tile([2, 1024], F32, tag="dsc", name="dsc")
                drec = nrm.tile([P, 16], F32, tag="drec", name="drec")
                for i, row in enumerate((D, 64 + D)):
                    nc.sync.dma_start(out=dsc[i:i + 1, :],
                                      in_=avf[row:row + 1, cl:ch])
                    nc.sync.dma_start(
                        out=drec[:, 8 * i:8 * (i + 1)],
                        in_=dsc[i, :].rearrange("(p f) -> p f", p=P))

                def mk(cl=cl, ch=ch, avf=avf, bc=bc, pair=pair, drec=drec):
                    def go():
                        dsc2 = dscp.tile([2, 1024], F32, tag="ds2",
                                         name="dsc2")
                        nc.vector.reciprocal(out=drec[:, :], in_=drec[:, :])
                        for i in range(2):
                            nc.sync.dma_start(
                                out=dsc2[i, :].rearrange("(p f) -> p f", p=P),
                                in_=drec[:, 8 * i:8 * (i + 1)])
                        for i, base in enumerate((0, 64)):
                            nc.sync.dma_start(
                                out=bc[base:base + D, cl:ch],
                                in_=dsc2[i, :].partition_broadcast(D))
                            # SBUF-only multiply -> GpSimd keeps DVE free
                            nc.gpsimd.tensor_mul(
                                nout[pair][base:base + D, cl:ch],
                                avf[base:base + D, cl:ch],
                                bc[base:base + D, cl:ch])
                    return go
                if not last:
                    deferred.append(mk())
                else:
                    mk()()
        # run the remaining deferred normalization (pair 3 half 0)
        for go in deferred:
            go()

    # ---- Phase 4: output projection (bias pre-folded into wp row 32 via
    # the ones-row in nout[0]) --------------------------------------------
    with tc.tile_pool(name="prp", bufs=4, space="PSUM") as prp, \
         tc.tile_pool(name="resp", bufs=1) as resp:
        resbig = resp.tile([P, NT, C], F32, name="resbig")
        out_r = out_d.rearrange("(n p) c -> n p c", p=P)
        # batched output DMAs, tapering so the last transfer is small
        flush_at = {3: 0, 7: 4, 11: 8, 13: 12, 15: 14}
        for n in range(NT):
            rp = prp.tile([P, C], F32, tag="rp", name="rp")
            for q in range(4):
                nc.tensor.matmul(
                    rp,
                    lhsT=nout[q][:, P * n:P * (n + 1)],
                    rhs=wp_sb[:, q, :],
                    start=(q == 0), stop=(q == 3))
            nc.vector.tensor_copy(resbig[:, n, :], rp)
            if n in flush_at:
                lo = flush_at[n]
                nc.scalar.dma_start(
                    out=out_r[lo:n + 1].rearrange("n p c -> p n c"),
                    in_=resbig[:, lo:n + 1, :])


def declare_io(nc):
    """Declare the kernel's DRAM IO tensors (shared with test harnesses)."""
    x_d = nc.dram_tensor("x", [C, T], BF16, kind="ExternalInput")
    wq_d = nc.dram_tensor("wq", [C, C], BF16, kind="ExternalInput")
    wk_d = nc.dram_tensor("wk", [C, C], BF16, kind="ExternalInput")
    wv_d = nc.dram_tensor("wv", [C, C], BF16, kind="ExternalInput")
    wp_d = nc.dram_tensor("wp", [4, P, C], BF16, kind="ExternalInput")
    bias_d = nc.dram_tensor("bias", [P, C], F32, kind="ExternalInput")
    out_d = nc.dram_tensor("out", [T, C], F32, kind="ExternalOutput")
    return x_d, wq_d, wk_d, wv_d, wp_d, bias_d, out_d


def build_nc():
    nc = bacc.Bacc("TRN2", debug=False, num_devices=N_CORES)
    x_d, wq_d, wk_d, wv_d, wp_d, bias_d, out_d = declare_io(nc)
    with tile.TileContext(nc) as tc:
        with ExitStack() as ctx:
            _body(nc, tc, ctx, x_d.ap(), wq_d.ap(), wk_d.ap(), wv_d.ap(),
                  wp_d.ap(), bias_d.ap(), out_d.ap())
    nc.compile()
    return nc


def prep_inputs(x, wq, wk, wv, wproj, bproj):
    """Host-side reformatting of the full inputs into per-core input maps."""
    f = np.float32
    bf = ml_dtypes.bfloat16
    # [H,C,D] -> [C, H*D]; wq additionally pre-scaled by 1/sqrt(C) (exact).
    wq2 = np.ascontiguousarray(
        np.transpose(np.asarray(wq, f), (1, 0, 2)).reshape(C, H * D)
        * f(1.0 / 16.0)).astype(bf)
    wk2 = np.ascontiguousarray(
        np.transpose(np.asarray(wk, f), (1, 0, 2)).reshape(C, H * D)).astype(bf)
    wv2 = np.ascontiguousarray(
        np.transpose(np.asarray(wv, f), (1, 0, 2)).reshape(C, H * D)).astype(bf)
    # wproj [H*D, C] -> 4 pair-chunks padded to 128 rows:
    # rows 0-31 <- head 2p, rows 64-95 <- head 2p+1, rest zero.
    wp4 = np.zeros((4, P, C), f)
    wproj = np.asarray(wproj, f)
    for p in range(4):
        wp4[p, 0:D] = wproj[64 * p: 64 * p + D]
        wp4[p, 64:64 + D] = wproj[64 * p + D: 64 * p + 2 * D]
    # bias rides row 32 of chunk 0 (ones-row in nout[0] row 32 on-device)
    wp4[0, D] = np.asarray(bproj, f)
    wp4 = wp4.astype(bf)
    bias128 = np.ascontiguousarray(
        np.broadcast_to(np.asarray(bproj, f), (P, C)))
    x_bf = np.asarray(x, f).astype(ml_dtypes.bfloat16)
    in_maps = []
    for b in range(N_CORES):
        in_maps.append({
            "x": np.ascontiguousarray(x_bf[b].T),  # host-side transpose
            "wq": wq2, "wk": wk2, "wv": wv2,
            "wp": wp4, "bias": bias128,
        })
    return in_maps


_NC_CACHE = []


def kernel(x, wq, wk, wv, wproj, bproj, _nc=None):
    in_maps = prep_inputs(x, wq, wk, wv, wproj, bproj)
    if _nc is None:
        if not _NC_CACHE:
            _NC_CACHE.append(build_nc())
        _nc = _NC_CACHE[0]
    res = run_bass_kernel_spmd(_nc, in_maps, list(range(N_CORES)))
    return np.stack([r["out"] for r in res.results], axis=0)



# revision 29
# speedup vs baseline: 1.1747x; 1.0717x over previous
"""Multi-head attention Bass kernel for Trainium2 (8 NeuronCores).

Problem: B=8, T=2048, C=256, H=8, D=32 MHA (dense, full softmax over T).
Sharding: data-parallel over batch -- core b computes batch b end-to-end,
no collectives.  Weights are replicated; per-core x slice is [T, C].

Per-core dataflow (v8):
  1. x arrives pre-transposed/pre-cast from the host as xT [C,T] bf16
     (plain DMAs, no on-device transpose).  wq pre-scaled by 1/sqrt(C)
     host-side; bias folded into wproj row 32 (ones-row trick).
  2. qT/kT [D,T] packed 4 heads per [128,T] tile via M=128 matmuls; v
     with an appended ones-column (v_ext [T,33], bf16) whose AV row
     yields the softmax denominators for free.
  3. Attention runs as 8 (pair, t-half) s-loops.  PSUM: av [P,1024] =
     2 banks + 3x sc [128,1024] = 6 banks.  The depth-3 scores pipeline
     feeds TWO exp engines concurrently -- ScalarE exact exp() for head A,
     DVE Schraudolph exp for head B (tensor_scalar int16(x*128/ln2 +
     16248) bit-cast to bf16; ~1.5% weight err, ~1e-2 output rel err
     total).  This dual drain is the key: a single-engine exp stream
     measures 2x slower end-to-end.  AV matmuls trail the scores by two
     s-iterations so the PE never stalls on the freshest exp chunk.
  4. Normalization: the [1,1024] denominator rows are reshaped to
     [128,16] via a DRAM hop and reciprocal'd WIDE on DVE (~0.2us; a
     single-lane [1,1024] DVE reciprocal costs ~20us on HW and wrecks
     the exp stream), shipped back, partition-broadcast by DMA, and
     multiplied into nout on the otherwise-idle GpSimd.  The inbound
     DMAs issue at the boundary; the reciprocal and outbound hops are
     deferred into the NEXT half's s-loop (popped at s==5), so only the
     two PSUM-freeing copies sit at the boundary.
  5. Projection: res = sum_pairs noutT_p.T @ wproj_p (bias rides the
     ones-row in nout[0]); output DMAs batched on the ScalarE HWDGE
     queue.
"""

import numpy as np
import ml_dtypes
from contextlib import ExitStack

import concourse.bass as bass
import concourse.bacc as bacc
import concourse.mybir as mybir
import concourse.tile as tile
from concourse.bass_utils import run_bass_kernel_spmd

B, T, C, H, D = 8, 2048, 256, 8, 32
P = 128
NT = T // P  # 16 chunks of 128 along t / s
F32 = mybir.dt.float32
BF16 = mybir.dt.bfloat16
I16 = mybir.dt.int16
EXP = mybir.ActivationFunctionType.Exp
AL = mybir.AluOpType
N_CORES = 8
E = D + 1  # 33: v columns + ones column

# Schraudolph exp on DVE: bf16 bits of exp(x) ~= int16(x*128/ln2 + 127*128-c).
# HW convert rounds to nearest (CoreSim truncates; both ~1.5% rel err on the
# weights, ~9.6e-3 on the output at a 50% chunk share).
_LN2 = float(np.log(2.0))
SCH_A = 128.0 / _LN2
SCH_B = 127.0 * 128.0 - 8.0


def _body(nc, tc, ctx, x_d, wq_d, wk_d, wv_d, wp_d, bias_d, out_d):
    const = ctx.enter_context(tc.tile_pool(name="const", bufs=1))
    big = ctx.enter_context(tc.tile_pool(name="big", bufs=1))

    wq_sb = const.tile([P, 2, C], BF16)
    wk_sb = const.tile([P, 2, C], BF16)
    wv_sb = const.tile([P, 2, C], BF16)
    wp_sb = const.tile([P, 4, C], BF16)
    ones_sb = const.tile([P, D], BF16)
    warm = const.tile([P, 1], F32)

    xT = [big.tile([P, T], BF16, name=f"xT{i}") for i in range(2)]
    qT = [big.tile([P, T], BF16, name=f"qT{i}") for i in range(2)]
    kT = [big.tile([P, T], BF16, name=f"kT{i}") for i in range(2)]
    v_sb = big.tile([P, NT, E * H], BF16)
    nout = [big.tile([P, T], BF16, name=f"nout{i}") for i in range(4)]

    # ---- Phase 1: x arrives pre-transposed from the host as xT [C,T];
    # plain DMAs only.  bf16 weights DMA'd straight into SBUF -------------
    for cc, eng in ((0, nc.scalar), (1, nc.sync)):
        eng.dma_start(out=xT[cc], in_=x_d[cc * P:(cc + 1) * P, :])
    for w_sb, w_d, nk in ((wq_sb, wq_d, 2), (wk_sb, wk_d, 2),
                          (wv_sb, wv_d, 2), (wp_sb, wp_d, 4)):
        if nk == 2:
            nc.scalar.dma_start(
                out=w_sb, in_=w_d.rearrange("(k p) c -> p k c", p=P))
        else:
            nc.scalar.dma_start(out=w_sb, in_=w_d.rearrange("q p c -> p q c"))
    nc.gpsimd.memset(v_sb, 1.0)  # ones cols survive; v overwrites the rest
    for t_ in nout:  # rows 32-63 / 96-127 must be 0.0 for the projection
        nc.gpsimd.memset(t_, 0.0)
    # ones-row at nout[0] row 32 picks up the bias row folded into wp[0]
    nc.vector.memset(nout[0][D:D + 1, :], 1.0)
    nc.vector.memset(ones_sb, 1.0)
    nc.scalar.activation(out=warm, in_=ones_sb[:, 0:1], func=EXP)

    # ---- Phase 2: v (warms the PE), then qT / kT (M=128 matmuls) --------
    with tc.tile_pool(name="pv", bufs=2, space="PSUM") as pv:
        for n in range(NT):
            vp = pv.tile([P, C], F32, tag="vp", name="vp")
            for cc in range(2):
                nc.tensor.matmul(
                    vp,
                    lhsT=xT[cc][:, n * P:(n + 1) * P],
                    rhs=wv_sb[:, cc, :],
                    start=(cc == 0), stop=(cc == 1))
            nc.vector.tensor_copy(
                v_sb[:, n].rearrange("p (h e) -> p h e", e=E)[:, :, 0:D],
                vp.rearrange("p (h d) -> p h d", d=D))
    with tc.tile_pool(name="pq", bufs=2, space="PSUM") as pq:
        for g in range(2):
            for w_sb, dest in ((wq_sb, qT), (wk_sb, kT)):
                qp = pq.tile([P, T], F32, tag="qp", name="qp")
                for ts in range(4):
                    for cc in range(2):
                        nc.tensor.matmul(
                            qp[:, 512 * ts:512 * (ts + 1)],
                            lhsT=w_sb[:, cc, P * g:P * (g + 1)],
                            rhs=xT[cc][:, 512 * ts:512 * (ts + 1)],
                            start=(cc == 0), stop=(cc == 1))
                if g == 0:  # ScalarE is idle pre-attention: fastest start
                    nc.scalar.copy(dest[g], qp)
                else:
                    nc.vector.tensor_copy(dest[g], qp)

    # ---- Phase 3: attention; (pair, t-half) loops ------------------------
    # PSUM: av [P,1024] = 2 banks (one t-half) + 3x sc [P,1024] = 6 banks.
    # The depth-3 scores pipeline lets ScalarE (exact exp, head A) and DVE
    # (Schraudolph int16 exp, head B) drain chunks concurrently; AV matmuls
    # trail the scores by a full s-iteration so the PE never stalls on the
    # freshest exp.  Normalization of each half is deferred into the next
    # half's s-loop (only the PSUM-freeing copies happen at the boundary).
    with tc.tile_pool(name="scp", bufs=3, space="PSUM") as scp, \
         tc.tile_pool(name="avp", bufs=1, space="PSUM") as avp, \
         tc.tile_pool(name="expp", bufs=10) as expp, \
         tc.tile_pool(name="nrm", bufs=2) as nrm, \
         tc.tile_pool(name="dsc", bufs=4, space="DRAM") as dscp:
        deferred = []

        def sc_chunk(g, h, half, s, use_dve):
            j = h % 4
            sc = scp.tile([P, 1024], F32, tag="sc", name="sc")
            for ts in range(2):
                tofs = 1024 * half + 512 * ts
                nc.tensor.matmul(
                    sc[:, 512 * ts:512 * (ts + 1)],
                    lhsT=kT[g][D * j:D * (j + 1), P * s:P * (s + 1)],
                    rhs=qT[g][D * j:D * (j + 1), tofs:tofs + 512],
                    start=True, stop=True,
                    tile_position=(D * j, 0))
            ex = expp.tile([P, 1024], BF16, tag="ex", name="ex")
            if use_dve:
                nc.vector.tensor_scalar(
                    ex[:, :].bitcast(I16), sc, SCH_A, SCH_B, AL.mult, AL.add)
            else:
                nc.scalar.activation(out=ex, in_=sc, func=EXP)
            return ex

        for pair in range(4):
            g = pair // 2
            hA, hB = 2 * pair, 2 * pair + 1
            avf = nrm.tile([P, T], F32, tag="avf", name="avf")
            bc = nrm.tile([P, T], F32, tag="bc", name="bc")
            for half in range(2):
                last = (pair == 3 and half == 1)
                cl, ch = 1024 * half, 1024 * (half + 1)
                av = avp.tile([P, 1024], F32, tag="av", name="av")

                def av_mm(s, exs):
                    for h in (hA, hB):
                        col = 0 if h == hA else 64
                        ex = exs.pop((s, h))
                        for ts in range(2):
                            # A and B share banks at different partition
                            # ranges; per-element has_written makes that safe
                            # on HW, but the sim's bank-granular group
                            # tracker would flag it.
                            nc.tensor.matmul(
                                av[col:col + E, 512 * ts:512 * (ts + 1)],
                                lhsT=v_sb[:, s, E * h:E * h + E],
                                rhs=ex[:, 512 * ts:512 * (ts + 1)],
                                start=(s == 0), stop=(s == NT - 1),
                                tile_position=(0, col),
                                skip_group_check=True)

                exs = {}
                for h, dve in ((hA, False), (hB, True)):
                    exs[(0, h)] = sc_chunk(g, h, half, 0, dve)
                for s in range(NT):
                    if s + 1 < NT:
                        for h, dve in ((hA, False), (hB, True)):
                            exs[(s + 1, h)] = sc_chunk(g, h, half, s + 1, dve)
                    if s >= 2:
                        av_mm(s - 2, exs)
                    if s == 5 and deferred:
                        deferred.pop(0)()
                av_mm(NT - 2, exs)
                av_mm(NT - 1, exs)

                # Evacuate the AV banks promptly: low rows on ScalarE,
                # high rows on DVE, in parallel.
                nc.scalar.copy(avf[0:E, cl:ch], av[0:E, :])
                nc.vector.tensor_copy(avf[64:64 + E, cl:ch],
                                      av[64:64 + E, :])

                # A [1,1024] DVE reciprocal is a single-lane iterative divide
                # (~20us on HW!), so reshape the two denominator rows to
                # [128,16] via a DRAM hop, take the reciprocal wide (~0.2us),
                # and ship back.  The inbound DMAs are issued here at the
                # boundary (sync queue, block nothing); only the reciprocal
                # and the outbound hops are deferred into the next half's
                # s-loop so the DVE never waits on the DMA chain.
                dsc = dscp.tile([2, 1024], F32, tag="dsc", name="dsc")
                drec = nrm.tile([P, 16], F32, tag="drec", name="drec")
                for i, row in enumerate((D, 64 + D)):
                    nc.sync.dma_start(out=dsc[i:i + 1, :],
                                      in_=avf[row:row + 1, cl:ch])
                    nc.sync.dma_start(
                        out=drec[:, 8 * i:8 * (i + 1)],
                        in_=dsc[i, :].rearrange("(p f) -> p f", p=P))

                def mk(cl=cl, ch=ch, avf=avf, bc=bc, pair=pair, drec=drec):
                    def go():
                        dsc2 = dscp.tile([2, 1024], F32, tag="ds2",
                                         name="dsc2")
                        nc.vector.reciprocal(out=drec[:, :], in_=drec[:, :])
                        for i in range(2):
                            nc.sync.dma_start(
                                out=dsc2[i, :].rearrange("(p f) -> p f", p=P),
                                in_=drec[:, 8 * i:8 * (i + 1)])
                        for i, base in enumerate((0, 64)):
                            nc.sync.dma_start(
                                out=bc[base:base + D, cl:ch],
                                in_=dsc2[i, :].partition_broadcast(D))
                            # SBUF-only multiply -> GpSimd keeps DVE free
                            nc.gpsimd.tensor_mul(
                                nout[pair][base:base + D, cl:ch],
                                avf[base:base + D, cl:ch],
                                bc[base:base + D, cl:ch])
                    return go
                if not last:
                    deferred.append(mk())
                else:
                    mk()()
        # run the remaining deferred normalization (pair 3 half 0)
        for go in deferred:
            go()

    # ---- Phase 4: output projection (bias pre-folded into wp row 32 via
    # the ones-row in nout[0]) --------------------------------------------
    with tc.tile_pool(name="prp", bufs=4, space="PSUM") as prp, \
         tc.tile_pool(name="resp", bufs=1) as resp:
        resbig = resp.tile([P, NT, C], F32, name="resbig")
        out_r = out_d.rearrange("(n p) c -> n p c", p=P)
        # batched output DMAs, tapering so the last transfer is small
        flush_at = {3: 0, 7: 4, 11: 8, 13: 12, 15: 14}
        for n in range(NT):
            rp = prp.tile([P, C], F32, tag="rp", name="rp")
            for q in range(4):
                nc.tensor.matmul(
                    rp,
                    lhsT=nout[q][:, P * n:P * (n + 1)],
                    rhs=wp_sb[:, q, :],
                    start=(q == 0), stop=(q == 3))
            nc.vector.tensor_copy(resbig[:, n, :], rp)
            if n in flush_at:
                lo = flush_at[n]
                nc.scalar.dma_start(
                    out=out_r[lo:n + 1].rearrange("n p c -> p n c"),
                    in_=resbig[:, lo:n + 1, :])


def declare_io(nc):
    """Declare the kernel's DRAM IO tensors (shared with test harnesses)."""
    x_d = nc.dram_tensor("x", [C, T], BF16, kind="ExternalInput")
    wq_d = nc.dram_tensor("wq", [C, C], BF16, kind="ExternalInput")
    wk_d = nc.dram_tensor("wk", [C, C], BF16, kind="ExternalInput")
    wv_d = nc.dram_tensor("wv", [C, C], BF16, kind="ExternalInput")
    wp_d = nc.dram_tensor("wp", [4, P, C], BF16, kind="ExternalInput")
    bias_d = nc.dram_tensor("bias", [P, C], F32, kind="ExternalInput")
    out_d = nc.dram_tensor("out", [T, C], F32, kind="ExternalOutput")
    return x_d, wq_d, wk_d, wv_d, wp_d, bias_d, out_d


def build_nc():
    nc = bacc.Bacc("TRN2", debug=False, num_devices=N_CORES)
    x_d, wq_d, wk_d, wv_d, wp_d, bias_d, out_d = declare_io(nc)
    with tile.TileContext(nc) as tc:
        with ExitStack() as ctx:
            _body(nc, tc, ctx, x_d.ap(), wq_d.ap(), wk_d.ap(), wv_d.ap(),
                  wp_d.ap(), bias_d.ap(), out_d.ap())
    nc.compile()
    return nc


def prep_inputs(x, wq, wk, wv, wproj, bproj):
    """Host-side reformatting of the full inputs into per-core input maps."""
    f = np.float32
    bf = ml_dtypes.bfloat16
    # [H,C,D] -> [C, H*D]; wq additionally pre-scaled by 1/sqrt(C) (exact).
    wq2 = np.ascontiguousarray(
        np.transpose(np.asarray(wq, f), (1, 0, 2)).reshape(C, H * D)
        * f(1.0 / 16.0)).astype(bf)
    wk2 = np.ascontiguousarray(
        np.transpose(np.asarray(wk, f), (1, 0, 2)).reshape(C, H * D)).astype(bf)
    wv2 = np.ascontiguousarray(
        np.transpose(np.asarray(wv, f), (1, 0, 2)).reshape(C, H * D)).astype(bf)
    # wproj [H*D, C] -> 4 pair-chunks padded to 128 rows:
    # rows 0-31 <- head 2p, rows 64-95 <- head 2p+1, rest zero.
    wp4 = np.zeros((4, P, C), f)
    wproj = np.asarray(wproj, f)
    for p in range(4):
        wp4[p, 0:D] = wproj[64 * p: 64 * p + D]
        wp4[p, 64:64 + D] = wproj[64 * p + D: 64 * p + 2 * D]
    # bias rides row 32 of chunk 0 (ones-row in nout[0] row 32 on-device)
    wp4[0, D] = np.asarray(bproj, f)
    wp4 = wp4.astype(bf)
    bias128 = np.ascontiguousarray(
        np.broadcast_to(np.asarray(bproj, f), (P, C)))
    x_bf = np.asarray(x, f).astype(ml_dtypes.bfloat16)
    in_maps = []
    for b in range(N_CORES):
        in_maps.append({
            "x": np.ascontiguousarray(x_bf[b].T),  # host-side transpose
            "wq": wq2, "wk": wk2, "wv": wv2,
            "wp": wp4, "bias": bias128,
        })
    return in_maps


_NC_CACHE = []


def kernel(x, wq, wk, wv, wproj, bproj, _nc=None):
    in_maps = prep_inputs(x, wq, wk, wv, wproj, bproj)
    if _nc is None:
        if not _NC_CACHE:
            _NC_CACHE.append(build_nc())
        _nc = _NC_CACHE[0]
    res = run_bass_kernel_spmd(_nc, in_maps, list(range(N_CORES)))
    return np.stack([r["out"] for r in res.results], axis=0)

